# revision 20
# baseline (speedup 1.0000x reference)
"""Trainium2 Bass kernel for nn_MultiHeadAttention_68659347194437.

Spatial multi-head attention over the W axis (no softmax):
    qp = wq*q + bq ; kp, vp likewise            (1x1 conv over C=32)
    attn = qp @ kp^T  per (b,h)                 [512, 512]
    att  = attn @ vp                            [512, 32]
    out  = att^T + q                            (NCHW residual)

Because there is no softmax, associativity collapses the [512,512] score
matrix:  out = L^T @ Q_aug per head, where
    G   = K_aug @ V_aug^T                [33,33]  (K_aug = [K; ones])
    A   = [wq|bq]^T [wk|bk]              [33,33]  (constant)
    L   = A G Pv^T + [I;0]               [33,32]  (Pv = [wv|bv]; +I = residual)
All projections, biases, and the residual fold into tiny per-head matmuls.

Sharding: data-parallel over batch B=8 across 8 NeuronCores, no comms.
Host-side work is pure relayout (transpose/concat/cast) of inputs.
"""

import os
import numpy as np

import concourse.bass as bass
import concourse.bacc as bacc
import concourse.tile as tile
import concourse.mybir as mybir
from concourse.bass_utils import run_bass_kernel_spmd

B, C, H, W = 8, 32, 64, 512
CA = C + 1          # augmented channel dim (ones row/col)
HW = H * W          # 32768 pixels per (b)
NCHUNK = HW // 128  # 256 chunks of 128 pixels (4 per head)
GH = 16             # heads per buffered group
NG = H // GH        # 8 groups

# dtype knobs: "f32" or "bf16" for the two big streaming paths
KV_DT = os.environ.get("KERNEL_KV_DT", "bf16")
Q_DT = os.environ.get("KERNEL_Q_DT", "bf16")

_DT = {"f32": mybir.dt.float32, "bf16": mybir.dt.bfloat16}

# exec time (ns) of the most recent run, when tracing was enabled
last_exec_time_ns = None

_cache = {}


def _np_dt(name):
    return np.dtype(np.float32) if name == "f32" else np.dtype(mybir.dt.np(mybir.dt.bfloat16))


def _build(kv_dt_name, q_dt_name):
    kv_dt = _DT[kv_dt_name]
    q_dt = _DT[q_dt_name]
    f32 = mybir.dt.float32

    nc = bacc.Bacc(
        "TRN2",
        target_bir_lowering=False,
        debug=False,
        enable_asserts=False,
        num_devices=8,
    )

    qa_d = nc.dram_tensor("qa", [CA, HW], q_dt, kind="ExternalInput")
    kt_d = nc.dram_tensor("kta", [128, NCHUNK * CA], kv_dt, kind="ExternalInput")
    vt_d = nc.dram_tensor("vta", [128, NCHUNK * CA], kv_dt, kind="ExternalInput")
    wqkb_d = nc.dram_tensor("wqkb", [C, 2 * CA], f32, kind="ExternalInput")  # [wq|bq|wk|bk]
    pvt_d = nc.dram_tensor("pvt", [CA, C], kv_dt, kind="ExternalInput")  # [wv^T; bv]
    out_d = nc.dram_tensor("out", [128, H // 4, W], f32, kind="ExternalOutput")

    # NEFF-embedded constants
    kv_np = _np_dt(kv_dt_name)
    i33_np = np.eye(CA, dtype=np.float32)
    iext_np = np.concatenate([np.eye(C), np.zeros((C, 1))], axis=1).astype(kv_np)
    i32_np = np.tile(np.eye(C), (1, 4)).astype(kv_np)  # [32, 128] tiled identity
    i33_d = nc.inline_tensor(i33_np, name="i33")
    iext_d = nc.inline_tensor(iext_np, name="iext")
    i32_d = nc.inline_tensor(i32_np, name="i32t")

    qa = qa_d.ap()
    kta = kt_d.ap()
    vta = vt_d.ap()
    out_ap = out_d.ap().rearrange("p a w -> p (a w)")

    with tile.TileContext(nc) as tc:
        with (
            tc.tile_pool(name="const", bufs=1) as cpool,
            tc.tile_pool(name="qin", bufs=4) as qpool,
            tc.tile_pool(name="kvin", bufs=4) as kvpool,
            tc.tile_pool(name="outp", bufs=3) as opool,
            tc.tile_pool(name="small", bufs=3) as spool,
            tc.tile_pool(name="psg", bufs=2, space=bass.MemorySpace.PSUM) as psg,
            tc.tile_pool(name="psy", bufs=2, space=bass.MemorySpace.PSUM) as psy,
            tc.tile_pool(name="psl", bufs=2, space=bass.MemorySpace.PSUM) as psl,
            tc.tile_pool(name="pso", bufs=2, space=bass.MemorySpace.PSUM) as pso,
        ):
            # ---- constants ----
            wqkb = cpool.tile([C, 2 * CA], f32)
            pvt = cpool.tile([CA, C], kv_dt)
            i33 = cpool.tile([CA, CA], f32)
            iext = cpool.tile([C, CA], kv_dt)
            i32 = cpool.tile([C, 4 * C], kv_dt)
            nc.scalar.dma_start(wqkb[:], wqkb_d.ap()[:])
            nc.scalar.dma_start(pvt[:], pvt_d.ap()[:])
            nc.sync.dma_start(i33[:], i33_d.ap()[:])
            nc.sync.dma_start(iext[:], iext_d.ap()[:])
            nc.sync.dma_start(i32[:], i32_d.ap()[:])
            wqb = wqkb[:, :CA]
            wkb = wqkb[:, CA:]

            # A = [wq|bq]^T [wk|bk]; AT = A^T via PE transpose with identity
            a_ps = psg.tile([CA, CA], f32, tag="g")
            nc.tensor.matmul(a_ps[:], wqb, wkb)
            a_sb = cpool.tile([CA, CA], f32)
            nc.vector.tensor_copy(a_sb[:], a_ps[:])
            at_ps = psg.tile([CA, CA], f32, tag="g")
            nc.tensor.matmul(at_ps[:], a_sb[:], i33[:])
            at_sb = cpool.tile([CA, CA], kv_dt)
            nc.vector.tensor_copy(at_sb[:], at_ps[:])

            # ---- main loop ----
            # leading groups are small so compute starts as soon as possible
            group_sizes = [4, 4, 8, 16, 16, 8, 8]
            assert sum(group_sizes) == H
            h0 = 0
            for g, ghn in enumerate(group_sizes):
                qg = qpool.tile([CA, ghn * W], q_dt, tag="qg")
                ktg = kvpool.tile([128, ghn * 4 * CA], kv_dt, tag="ktg")
                vtg = kvpool.tile([128, ghn * 4 * CA], kv_dt, tag="vtg")
                og = opool.tile([128, (ghn // 4) * W], f32, tag="og")

                nc.sync.dma_start(ktg[:], kta[:, h0 * 4 * CA:(h0 + ghn) * 4 * CA])
                nc.scalar.dma_start(vtg[:], vta[:, h0 * 4 * CA:(h0 + ghn) * 4 * CA])
                nc.scalar.dma_start(qg[:], qa[:, h0 * W:(h0 + ghn) * W])

                for blk in range(ghn // 4):
                    m1b = spool.tile([CA, 4 * C], kv_dt, tag="m1b")
                    # GT for 4 heads share one PSUM bank; one copy for all 4
                    gt_ps = psg.tile([CA, 4 * CA], f32, tag="g")
                    gt_sb = spool.tile([CA, 4 * CA], kv_dt, tag="gt_sb")
                    m1_ps = psy.tile([CA, 4 * C], f32, tag="m1")
                    for i in range(4):
                        hh = blk * 4 + i
                        # GT = V_aug K_aug^T, accumulated over 4 x-chunks
                        for j in range(4):
                            o = (hh * 4 + j) * CA
                            nc.tensor.matmul(
                                gt_ps[:, i * CA:(i + 1) * CA],
                                vtg[:, o:o + CA],
                                ktg[:, o:o + CA],
                                start=(j == 0),
                                stop=(j == 3),
                            )
                    nc.any.tensor_copy(gt_sb[:], gt_ps[:])
                    for i in range(4):
                        # M1 = G_aug Pv^T
                        nc.tensor.matmul(
                            m1_ps[:, i * C:(i + 1) * C],
                            gt_sb[:, i * CA:(i + 1) * CA],
                            pvt[:],
                        )
                    nc.any.tensor_copy(m1b[:], m1_ps[:])

                    # L = A M1 + [I;0] for 4 heads at once
                    l_ps = psl.tile([CA, 4 * C], f32, tag="l")
                    nc.tensor.matmul(l_ps[:], iext[:], i32[:], start=True, stop=False)
                    nc.tensor.matmul(l_ps[:], at_sb[:], m1b[:], start=False, stop=True)
                    l_sb = spool.tile([CA, 4 * C], q_dt, tag="l_sb")
                    nc.any.tensor_copy(l_sb[:], l_ps[:])

                    # out = L^T Q_aug, 4 heads col-tiled into one [128, W] bank
                    o_ps = pso.tile([128, W], f32, tag="o")
                    for i in range(4):
                        hh = blk * 4 + i
                        nc.tensor.matmul(
                            o_ps[32 * i:32 * (i + 1), :],
                            l_sb[:, i * C:(i + 1) * C],
                            qg[:, hh * W:(hh + 1) * W],
                            tile_position=(0, 32 * i),
                        )
                    if blk % 2 == 0:
                        nc.vector.tensor_copy(og[:, blk * W:(blk + 1) * W], o_ps[:])
                    else:
                        nc.scalar.copy(og[:, blk * W:(blk + 1) * W], o_ps[:])

                base = (h0 // 4) * W
                ow = (ghn // 4) * W
                if ghn > 8:
                    nc.sync.dma_start(out_ap[:, base:base + ow // 2], og[:, :ow // 2])
                    nc.scalar.dma_start(out_ap[:, base + ow // 2:base + ow], og[:, ow // 2:])
                else:
                    nc.sync.dma_start(out_ap[:, base:base + ow], og[:])
                h0 += ghn

    nc.compile()
    return nc


def _prep_core(qb, kb, vb, q_np_dt, kv_np_dt):
    """Host-side relayout for one batch element (one core)."""
    qa = np.empty((CA, HW), dtype=q_np_dt)
    qa[:C] = qb.reshape(C, HW)
    qa[C] = 1.0

    def tr(x):
        t = np.empty((HW, CA), dtype=np.float32)
        t[:, :C] = x.reshape(C, HW).T
        t[:, C] = 1.0
        return np.ascontiguousarray(
            t.reshape(NCHUNK, 128, CA).transpose(1, 0, 2)
        ).reshape(128, NCHUNK * CA).astype(kv_np_dt)

    return qa, tr(kb), tr(vb)


def _install_ntff_hook():
    """Provide antenv.axon_hooks (absent in this image) so trace=True works."""
    import sys
    import types

    if "antenv.axon_hooks" in sys.modules:
        return
    try:
        import antenv
    except ImportError:
        return
    mod = types.ModuleType("antenv.axon_hooks")
    store = {}
    mod.set_axon_ntff_profile_hook = lambda h: store.__setitem__("h", h)
    mod.get_axon_ntff_profile_hook = lambda: store.get("h")
    sys.modules["antenv.axon_hooks"] = mod
    antenv.axon_hooks = mod
    try:
        from trn_agent_boot.trn_boot import _ntff_profile_via_ctypes

        hook = _ntff_profile_via_ctypes("/opt/axon/libaxon_pjrt.so")
        if hook is not None:
            store["h"] = hook
    except Exception:
        pass


def kernel(q, k, v, wq, bq, wk, bk, wv, bv):
    global last_exec_time_ns
    key = (KV_DT, Q_DT)
    if key not in _cache:
        _cache[key] = _build(*key)
    nc = _cache[key]

    q_np_dt = _np_dt(Q_DT)
    kv_np_dt = _np_dt(KV_DT)

    q = np.asarray(q, np.float32)
    k = np.asarray(k, np.float32)
    v = np.asarray(v, np.float32)
    wq = np.asarray(wq, np.float32)
    bq = np.asarray(bq, np.float32)
    wk = np.asarray(wk, np.float32)
    bk = np.asarray(bk, np.float32)
    wv = np.asarray(wv, np.float32)
    bv = np.asarray(bv, np.float32)

    wqkb = np.concatenate([wq, bq[:, None], wk, bk[:, None]], axis=1)  # [32, 66]
    pvt = np.concatenate([wv.T, bv[None, :]], axis=0).astype(kv_np_dt)  # [33, 32]

    in_maps = []
    for b in range(B):
        qa, kta, vta = _prep_core(q[b], k[b], v[b], q_np_dt, kv_np_dt)
        in_maps.append({
            "qa": qa, "kta": kta, "vta": vta,
            "wqkb": wqkb, "pvt": pvt,
        })

    trace = os.environ.get("KERNEL_TRACE", "0") == "1"
    if trace:
        _install_ntff_hook()
    res = run_bass_kernel_spmd(nc, in_maps, core_ids=list(range(B)), trace=trace)
    last_exec_time_ns = res.exec_time_ns

    outs = []
    for b in range(B):
        arr = res.results[b]["out"].reshape(4, C, H // 4, W)
        outs.append(np.transpose(arr, (1, 2, 0, 3)).reshape(C, H, W))
    return np.stack(outs).astype(np.float32)
